# revision 81
# baseline (speedup 1.0000x reference)
"""Trainium2 Bass kernel for nn_MultiHeadAttention (B=4, T=1024, D=1024, H=16, dk=64).

Sharding: 8 cores = 4 batches x 2 head-groups (8 heads / 512 features each).
Each core computes a partial output (its head-group's contribution through Wo);
host sums the two partials per batch (the "all-reduce after linear_out" done
host-side during unshard) and adds bo.

Per-core dataflow (all on one NeuronCore, Tile-scheduled):
  A) q/k/v projections as X^T-major bf16 matmuls -> (Tq partitions, F free).
     Two-pass structure so the PE's in-order queue never stalls behind a
     tile's LayerNorm chain: pass1 = proj matmuls + ACT drain + per-head LN
     (bn_stats on DVE, even/odd aggregation, normalize on GPSIMD -> bf16
     qhat, one qhat tile per (proj, t) kept resident); pass2 = PE-transpose
     128x128 blocks into qlnT/klnT (F partitions, T free) with gamma (and
     1/sqrt(dk) for q) applied per-partition in ONE [P,512] drain per tile.
     Order: k in two d-major waves of 4 t-blocks (matmuls consume w/x
     d-chunks as the DMAs land, PE starts ~2us in), v tiles 0-2, dense
     q-pass1, then a fused loop of k-pass2 + v tiles 3-7 + q-pass2 whose
     LN chains completed under the preceding stretch. A PE warm-up burst
     releases the HAM clock gate before the first real matmul.
  B) per head: scoresT = klnT-slice.T @ qlnT-slice (K=64), exp on ACT (PSUM
     [128,1024] -> SBUF bf16, the phase-B bottleneck at ~1.04us/tile), mask
     multiply (bf16 DVE, full-T), x_aug = [v|1].T @ attnT accumulated over
     Tk chunks -> 64 rows of x + 64 rows of broadcast denominator. Each
     head's denominator tail (exact DVE reciprocal, DMA partition-shift,
     normalize into x_all bf16) is DEFERRED into the middle of the next
     head's tk loop so it rides DVE slack instead of stalling the
     exp->mask->attnV chain; the last tail shifts via a host-provided
     64-roll permutation matmul on the then-idle PE.
  C) out = x_all-slices.T @ WoT (bf16) -> (T, 1024) bf16 partial, ACT drain
     to SBUF (4 output bufs so the DMA latency never blocks PSUM recycling),
     DMA out.

All DRAM traffic is bf16 (inputs cast host-side, ~12 MB total vs 26 MB in
f32), which keeps the serialized DMA device well under the PE.

Toolchain constraints hit while building this (kept for posterity): walrus
allows only ONE sync-wait per instruction (_split_excess_waits patches the
BIR); batched multi-group bn_stats is rejected ("output must be 6
elements/partition"); Memset on f32r is invalid ISA (the roll matrix is a
DRAM input instead); matmul dst with a partition offset fails
s3d3_mm_valid_dst_partition (hence the roll-permutation shift, not
tile_position); Act Rsqrt/Reciprocal are blocked for accuracy; DVE
tensor_tensor may read at most one PSUM operand.
"""

import numpy as np
import ml_dtypes

T = 1024
D = 1024
F = 512      # features per core (8 heads x 64)
NH = 8       # heads per core
DK = 64
P = 128
EPS = 1e-5
BF16 = ml_dtypes.bfloat16

_CACHE = {}

# engine-assignment knobs
NORM_ENGINE = "gpsimd"   # qhat normalize: "gpsimd" | "vector"
GDRAIN = "act"           # gamma-drain post-transpose: "act" | "vector" | "alt"
SC_BUFS = 2
PS512_BUFS = 2
ATTN_BUFS = 6
DRAIN_BUFS = 4
STAT_BUFS = 4
QHAT_BUFS = 12
MASK_FULL_T = True       # single [P,1024] mask multiply per (j,hh,tk)
OUT_DTYPE = "bf16"       # "bf16" | "f32"
PE_SHIFT_LAST = True     # last tail via PE identity shift
WARMUP_MM = 26           # dummy matmuls to release the PE clock gate
V_ILV = False             # interleave v_proj into first b_pair
TAIL_DEFER = True        # defer denom tails into next head group


def _split_excess_waits(bj):
    """Walrus allows at most 1 sync-wait per instruction (2 for
    EventSemaphore). Tile's sem assigner can emit more; spill the excess
    onto NoOp carriers inserted just before, on the same engine."""
    import json
    d = json.loads(bj)
    ctr = 0
    for fn in d["functions"]:
        for bb in fn["blocks"]:
            new = []
            for inst in bb["instructions"]:
                si = inst.get("sync_info") or {}
                ow = si.get("on_wait") or []
                op = inst.get("opcode", "")
                cap = 2 if op == "EventSemaphore" else 1
                if len(ow) > cap:
                    for w in ow[:-cap]:
                        ctr += 1
                        new.append({
                            "debug": inst.get("debug", 0),
                            "engine": inst["engine"],
                            "ins": [], "outs": [],
                            "name": f"W-{ctr}",
                            "opcode": "NoOp",
                            "sync_info": {"on_update": [], "on_wait": [w]},
                            "text_hint": "waitsplit",
                        })
                    si["on_wait"] = ow[-cap:]
                new.append(inst)
            bb["instructions"] = new
    return json.dumps(d).encode(), ctr


def _build(use_bq, use_bk, use_bv, ln_beta_zero=True):
    import concourse.bass as bass
    import concourse.tile as tile
    from concourse import mybir

    f32 = mybir.dt.float32
    f32r = mybir.dt.float32r
    bf16 = mybir.dt.bfloat16
    out_dt = bf16 if OUT_DTYPE == "bf16" else f32

    nc = bass.Bass()

    # ---- DRAM I/O (all big tensors bf16) ----
    xq_t = nc.dram_tensor("xq_t", (D, T), bf16, kind="ExternalInput").ap()
    xk_t = nc.dram_tensor("xk_t", (D, T), bf16, kind="ExternalInput").ap()
    xv_t = nc.dram_tensor("xv_t", (D, T), bf16, kind="ExternalInput").ap()
    wq_t = nc.dram_tensor("wq_t", (D, F), bf16, kind="ExternalInput").ap()
    wk_t = nc.dram_tensor("wk_t", (D, F), bf16, kind="ExternalInput").ap()
    wv_t = nc.dram_tensor("wv_t", (D, F), bf16, kind="ExternalInput").ap()
    wo_t = nc.dram_tensor("wo_t", (F, D), bf16, kind="ExternalInput").ap()
    mask_t = nc.dram_tensor("mask_t", (T, T), bf16, kind="ExternalInput").ap()
    # per-partition LN constants (128,) = per (head-pair-local feature)
    gq_d = nc.dram_tensor("gq", (P, 1), f32, kind="ExternalInput").ap()
    bq_d = nc.dram_tensor("bq_ln", (P, 1), f32, kind="ExternalInput").ap()
    gk_d = nc.dram_tensor("gk", (P, 1), f32, kind="ExternalInput").ap()
    bk_d = nc.dram_tensor("bk_ln", (P, 1), f32, kind="ExternalInput").ap()
    biases = {}
    for name, used in (("bq", use_bq), ("bk", use_bk), ("bv", use_bv)):
        if used:
            biases[name] = nc.dram_tensor(name, (F,), f32, kind="ExternalInput").ap()
    roll_d = nc.dram_tensor("roll_d", (P, P), f32r, kind="ExternalInput").ap()
    out_p = nc.dram_tensor("out_p", (T, D), out_dt, kind="ExternalOutput").ap()

    # DRAM views
    xviews = {
        "q": xq_t.rearrange("(dc p) t -> p dc t", p=P),
        "k": xk_t.rearrange("(dc p) t -> p dc t", p=P),
        "v": xv_t.rearrange("(dc p) t -> p dc t", p=P),
    }
    wviews = {
        "q": wq_t.rearrange("(dc p) f -> p dc f", p=P),
        "k": wk_t.rearrange("(dc p) f -> p dc f", p=P),
        "v": wv_t.rearrange("(dc p) f -> p dc f", p=P),
    }
    wo_view = wo_t.rearrange("(fc p) d -> p fc d", p=P)
    mask_view = mask_t.rearrange("(kc p) t -> p kc t", p=P)
    out_view = out_p.rearrange("(tc p) d -> p tc d", p=P)

    with tile.TileContext(nc) as tc:
        with (
            tc.tile_pool(name="const", bufs=1) as const,
            tc.tile_pool(name="drain", bufs=DRAIN_BUFS) as drain,
            tc.tile_pool(name="stat", bufs=STAT_BUFS) as stat,
            tc.tile_pool(name="qhatp", bufs=QHAT_BUFS) as qhatp,
            tc.tile_pool(name="attnp", bufs=ATTN_BUFS) as attnp,
            tc.tile_pool(name="recipp", bufs=3) as recipp,
            tc.tile_pool(name="outp", bufs=3) as outp,
            tc.tile_pool(name="ps512", bufs=PS512_BUFS, space="PSUM") as ps512,
            tc.tile_pool(name="ps1024", bufs=SC_BUFS, space="PSUM") as ps1024,
        ):
            # ---- resident tiles ----
            w_sb = {
                pn: const.tile([P, 8, F], bf16, name=f"w_{pn}", tag=f"w_{pn}")
                for pn in ("q", "k", "v")
            }
            x_sb = {
                pn: const.tile([P, 8, T], bf16, name=f"x_{pn}", tag=f"x_{pn}")
                for pn in ("q", "k", "v")
            }
            wo_sb = const.tile([P, 4, D], bf16, name="wo", tag="wo")
            qlnT = const.tile([P, 4, T], bf16, name="qlnT", tag="qlnT")
            klnT = const.tile([P, 4, T], bf16, name="klnT", tag="klnT")
            vaug = const.tile([P, 8, NH, P], bf16, name="vaug", tag="vaug")  # [p, tk, h, 128]
            mask_sb = const.tile([P, 8, T], bf16, name="mask", tag="mask")
            x_all = const.tile([P, 4, T], bf16, name="xall", tag="xall")
            eps_t = const.tile([P, 1], f32, name="eps", tag="eps")
            gb_t = {}

            # PE warm-up: ~3.4us of dummy matmuls releases the HAM clock
            # gate so the first real projections run at full rate
            warm = const.tile([P, P], bf16, name="warm", tag="warm")
            nc.vector.memset(warm, 0.0)
            wps = ps512.tile([P, F], f32, name="warmps", tag="ps512")
            for i in range(WARMUP_MM):
                nc.tensor.matmul(wps[:, 0:P], lhsT=warm, rhs=warm,
                                 start=(i == 0), stop=(i == WARMUP_MM - 1))

            # ---- startup DMA order: k-path first so PE starts ASAP ----
            # (w d-pair, x d-pair for the first 4 t-blocks) interleaved so the
            # d-major wave-1 matmuls start ~2us in; second-half columns follow
            for dl, dh in ((0, 2), (2, 4), (4, 6), (6, 8)):
                nc.sync.dma_start(w_sb["k"][:, dl:dh, :], wviews["k"][:, dl:dh, :])
                nc.sync.dma_start(x_sb["k"][:, dl:dh, 0:512],
                                  xviews["k"][:, dl:dh, 0:512])
            nc.sync.dma_start(x_sb["k"][:, 0:4, 512:1024],
                              xviews["k"][:, 0:4, 512:1024])
            nc.sync.dma_start(x_sb["k"][:, 4:8, 512:1024],
                              xviews["k"][:, 4:8, 512:1024])
            _dma2 = nc.gpsimd if SWDGE_LOADS else nc.sync
            _dma2.dma_start(w_sb["v"], wviews["v"])
            _dma2.dma_start(x_sb["v"][:, :, 0:512], xviews["v"][:, :, 0:512])
            _dma2.dma_start(w_sb["q"], wviews["q"])
            _dma2.dma_start(x_sb["q"][:, :, 0:512], xviews["q"][:, :, 0:512])
            _dma2.dma_start(x_sb["v"][:, :, 512:1024], xviews["v"][:, :, 512:1024])
            _dma2.dma_start(x_sb["q"][:, :, 512:1024], xviews["q"][:, :, 512:1024])
            _dma2.dma_start(mask_sb[:, 0:4, :], mask_view[:, 0:4, :])
            _dma2.dma_start(mask_sb[:, 4:8, :], mask_view[:, 4:8, :])
            _dma2.dma_start(wo_sb, wo_view)

            nc.vector.memset(eps_t, EPS)
            from concourse.masks import make_identity
            ident = const.tile([P, P], bf16, name="ident", tag="ident")
            make_identity(nc, ident)
            identr = const.tile([P, P], f32r, name="identr", tag="identr")

            def load_ln_consts():
                # emitted after the k projections start: keeps the Act queue
                # free for the first PSUM drains
                loads = [("gq", gq_d), ("gk", gk_d)]
                if not ln_beta_zero:
                    loads += [("bq", bq_d), ("bk", bk_d)]
                for nm, dr in loads:
                    gb_t[nm] = const.tile([P, 1], f32, name=f"ln_{nm}", tag=f"ln_{nm}")
                    nc.scalar.dma_start(gb_t[nm], dr)
                # 64-roll permutation (host-provided): shifts partitions by 64
                # in either direction via a standard K=64 matmul
                nc.scalar.dma_start(identr, roll_d)

            bias_bc = {}
            for name in biases:
                bias_bc[name] = const.tile([P, F], f32, name=f"bc_{name}", tag=f"bc_{name}")
                src = bass.AP(
                    tensor=biases[name].tensor,
                    offset=biases[name].offset,
                    ap=[[0, P], [1, F]],
                )
                nc.gpsimd.dma_start(out=bias_bc[name], in_=src)

            # ones columns of v_aug: even h -> cols 64:128, odd h -> cols 0:64
            nc.gpsimd.memset(vaug[:, :, 0::2, DK:P], 1.0)
            nc.gpsimd.memset(vaug[:, :, 1::2, 0:DK], 1.0)

            ln_params = {"q": ("gq", "bq"), "k": ("gk", "bk")}

            # ---- Phase A: projections + LN (pass1), transposes (pass2) ----
            # Transposes are split into a second pass so the PE's in-order
            # queue never stalls behind a tile's LayerNorm chain; v-projection
            # tiles are interleaved into the pass1 loops to fill PE slack
            # before attention starts.
            qh_tiles = {}

            def proj_pass1(pn, t, v_ts=(), k2_dst=None):
                ps = ps512.tile([P, F], f32, name="ps512", tag="ps512")
                for d in range(8):
                    nc.tensor.matmul(
                        ps, lhsT=x_sb[pn][:, d, t * P:(t + 1) * P],
                        rhs=w_sb[pn][:, d, :],
                        start=(d == 0), stop=(d == 7),
                    )
                proj_ln_chain(pn, t, ps)

            def proj_ln_chain(pn, t, ps):
                bias_name = "b" + pn
                if True:
                    sb = drain.tile([P, NH, DK], f32, name="qsb", tag="qsb")
                    if bias_name in bias_bc:
                        nc.vector.tensor_add(
                            sb.rearrange("p h d -> p (h d)"), ps, bias_bc[bias_name])
                    else:
                        nc.scalar.activation(
                            out=sb.rearrange("p h d -> p (h d)"), in_=ps,
                            func=mybir.ActivationFunctionType.Copy)
                    st = stat.tile([P, NH, 6], f32, name="st", tag="st")
                    for h in range(NH):
                        nc.vector.bn_stats(out=st[:, h, :], in_=sb[:, h, :])
                    # combine even/odd halves: mu=(me+mo)/2;
                    # var=(32ve+32vo)/64 + ((me-mo)/2)^2
                    me, mo = st[:, :, 1], st[:, :, 4]
                    ve, vo = st[:, :, 2], st[:, :, 5]
                    mu = stat.tile([P, NH], f32, name="mu", tag="mu")
                    nc.vector.tensor_add(mu, me, mo)
                    nc.vector.tensor_scalar_mul(mu, mu, 0.5)
                    dm = stat.tile([P, NH], f32, name="dm", tag="dm")
                    nc.vector.tensor_sub(dm, me, mo)
                    nc.vector.tensor_scalar_mul(dm, dm, 0.5)
                    nc.vector.tensor_mul(dm, dm, dm)  # ((me-mo)/2)^2
                    sv = stat.tile([P, NH], f32, name="sv", tag="sv")
                    nc.vector.tensor_add(sv, ve, vo)
                    var = stat.tile([P, NH], f32, name="var", tag="var")
                    # var = sv/64 + dm
                    nc.vector.scalar_tensor_tensor(
                        out=var, in0=sv, scalar=1.0 / DK, in1=dm,
                        op0=mybir.AluOpType.mult,
                        op1=mybir.AluOpType.add)
                    sd = stat.tile([P, NH], f32, name="sd", tag="sd")
                    nc.scalar.activation(
                        out=sd, in_=var,
                        func=mybir.ActivationFunctionType.Sqrt,
                        bias=eps_t,
                    )
                    rs = stat.tile([P, NH], f32, name="rs", tag="rs")
                    nc.vector.reciprocal(out=rs, in_=sd)
                    qh = qhatp.tile([P, F], bf16, name="qh", tag="qh")
                    # last q tiles gate the first attention scores: use the
                    # lower-latency DVE for their normalize
                    fast = (pn == "q" and t >= 7)
                    norm_eng = (nc.vector if (NORM_ENGINE == "vector" or fast)
                                else nc.gpsimd)
                    for h in range(NH):
                        norm_eng.tensor_scalar(
                            out=qh[:, h * DK:(h + 1) * DK],
                            in0=sb[:, h, :],
                            scalar1=mu[:, h:h + 1],
                            scalar2=rs[:, h:h + 1],
                            op0=mybir.AluOpType.subtract,
                            op1=mybir.AluOpType.mult,
                        )
                    qh_tiles[(pn, t)] = qh

            def proj_pass2(pn, t, dstT):
                qh = qh_tiles.pop((pn, t))
                g_nm, b_nm = ln_params[pn]
                pst = ps1024.tile([P, F], bf16, name="sc_bf", tag="sc")
                for j in range(4):
                    nc.tensor.transpose(
                        pst[:, j * P:(j + 1) * P], qh[:, j * P:(j + 1) * P], ident)
                # one drain for all 4 j-blocks: gamma is the same
                # per-partition vector for every head pair
                dst = dstT[:, 0:4, t * P:(t + 1) * P]
                if GDRAIN == "act":
                    deng = "act"
                elif GDRAIN == "vector":
                    deng = "vec"
                else:
                    deng = "act" if t % 2 == 0 else "vec"
                if ln_beta_zero:
                    if deng == "act":
                        nc.scalar.activation(
                            out=dst, in_=pst,
                            func=mybir.ActivationFunctionType.Copy,
                            scale=gb_t[g_nm],
                        )
                    else:
                        nc.vector.tensor_scalar_mul(dst, pst, gb_t[g_nm])
                else:
                    nc.vector.tensor_scalar(
                        out=dst, in0=pst,
                        scalar1=gb_t[g_nm], scalar2=gb_t[b_nm],
                        op0=mybir.AluOpType.mult, op1=mybir.AluOpType.add,
                    )

            def v_proj(ts_list):
                for t in ts_list:
                    ps = ps512.tile([P, F], f32, name="psv", tag="ps512")
                    for d in range(8):
                        nc.tensor.matmul(
                            ps, lhsT=x_sb["v"][:, d, t * P:(t + 1) * P],
                            rhs=w_sb["v"][:, d, :],
                            start=(d == 0), stop=(d == 7),
                        )
                    ps_h = ps.rearrange("p (hp two d) -> p hp two d", two=2, d=DK)
                    if "bv" in bias_bc:
                        vb = drain.tile([P, NH, DK], f32, name="vsb", tag="qsb")
                        nc.vector.tensor_add(
                            vb.rearrange("p h d -> p (h d)"), ps, bias_bc["bv"])
                        vb_h = vb.rearrange("p (hp two) d -> p hp two d", two=2)
                        nc.vector.tensor_copy(out=vaug[:, t, 0::2, 0:DK], in_=vb_h[:, :, 0, :])
                        nc.vector.tensor_copy(out=vaug[:, t, 1::2, DK:P], in_=vb_h[:, :, 1, :])
                    else:
                        nc.vector.tensor_copy(out=vaug[:, t, 0::2, 0:DK], in_=ps_h[:, :, 0, :])
                        nc.vector.tensor_copy(out=vaug[:, t, 1::2, DK:P], in_=ps_h[:, :, 1, :])

            # k pass1 (+ first v tiles once xv is in), q pass1 interleaved
            # with k's transposes and the remaining v tiles, then q transposes
            # k projection in two d-major waves of 4 t-blocks: wave-1
            # matmuls consume (w,x) d-chunks as the DMAs land
            for wave in range(2):
                ts = range(4 * wave, 4 * wave + 4)
                pss = {}
                for t in ts:
                    pss[t] = ps512.tile([P, F], f32, name="ps512", tag="ps512")
                for d in range(8):
                    for t in ts:
                        nc.tensor.matmul(
                            pss[t], lhsT=x_sb["k"][:, d, t * P:(t + 1) * P],
                            rhs=w_sb["k"][:, d, :],
                            start=(d == 0), stop=(d == 7),
                        )
                for t in ts:
                    proj_ln_chain("k", t, pss[t])
                if wave == 0:
                    load_ln_consts()
                else:
                    v_proj([0, 1, 2])
            for t in range(8):
                proj_pass1("q", t)
            for t in range(8):
                proj_pass2("k", t, klnT)
                if t >= 3:
                    v_proj([t])
                proj_pass2("q", t, qlnT)

            # ---- Phase B: attention ----
            # Each head's denominator tail (reciprocal -> partition shift ->
            # normalize) is deferred into the middle of the NEXT head's tk
            # loop: the DVE ops land in DVE idle slack instead of stalling the
            # exp->mask->attnV chain at the head boundary. The last tail uses
            # a PE identity-matmul shift (PE is idle at the B->C edge).
            pending_tail = []

            def b_tail(j, hh, xps, pe_shift):
                xrows = slice(0, DK) if hh == 0 else slice(DK, P)
                drows = slice(DK, P) if hh == 0 else slice(0, DK)
                rcs = []
                for c in range(2):
                    rc = recipp.tile([P, F], f32r, name="rc", tag="rc")
                    with nc.allow_low_precision(reason="f32r==f32 bits; recip of softmax denom"):
                        nc.vector.reciprocal(out=rc[drows], in_=xps[c][drows])
                    rcs.append(rc)
                if pe_shift:
                    for c in range(2):
                        # roll matrix maps partitions drows -> xrows
                        rps = ps1024.tile([P, T], f32, name="rps", tag="sc")[:, 0:F]
                        nc.tensor.matmul(
                            rps, lhsT=identr[drows, :], rhs=rcs[c][drows],
                            start=True, stop=True)
                        rsh = recipp.tile([P, F], f32r, name="rsh", tag="rsh")
                        nc.vector.tensor_copy(out=rsh[xrows], in_=rps[xrows])
                        nc.vector.tensor_mul(
                            x_all[xrows, j, c * F:(c + 1) * F],
                            xps[c][xrows], rsh[xrows])
                else:
                    rshs = []
                    for c in range(2):
                        rsh = recipp.tile([P, F], f32r, name="rsh", tag="rsh")
                        nc.sync.dma_start(out=rsh[xrows], in_=rcs[c][drows])
                        rshs.append(rsh)
                    for c in range(2):
                        nc.vector.tensor_mul(
                            x_all[xrows, j, c * F:(c + 1) * F],
                            xps[c][xrows], rshs[c][xrows])

            def b_pair_full(j, with_v=False, last=False):
                for hh in range(2):
                    h = 2 * j + hh
                    rows = slice(hh * DK, (hh + 1) * DK)
                    xps = [ps512.tile([P, F], f32, name="ps512", tag="ps512")
                           for _ in range(2)]
                    for tk in range(8):
                        if V_ILV and with_v and hh == 0 and tk < 6:
                            v_proj([tk + 2])
                        sp = ps1024.tile([P, T], f32, name="sc", tag="sc")
                        lt = klnT[rows, j, tk * P:(tk + 1) * P]
                        nc.tensor.matmul(sp[:, 0:F], lhsT=lt, rhs=qlnT[rows, j, 0:F],
                                         start=True, stop=True)
                        nc.tensor.matmul(sp[:, F:T], lhsT=lt, rhs=qlnT[rows, j, F:T],
                                         start=True, stop=True)
                        at = attnp.tile([P, T], bf16, name="attn_f", tag="attn")
                        nc.scalar.activation(
                            out=at, in_=sp, func=mybir.ActivationFunctionType.Exp)
                        nc.vector.tensor_mul(at, at, mask_sb[:, tk, :])
                        for c in range(2):
                            nc.tensor.matmul(
                                xps[c], lhsT=vaug[:, tk, h, :],
                                rhs=at[:, c * F:(c + 1) * F],
                                start=(tk == 0), stop=(tk == 7))
                        if tk == TAIL_FLUSH_TK and pending_tail:
                            pending_tail.pop(0)()
                    if last and hh == 1:
                        b_tail(j, hh, xps, pe_shift=PE_SHIFT_LAST)
                    elif TAIL_DEFER:
                        pending_tail.append(
                            lambda j=j, hh=hh, xps=xps:
                            b_tail(j, hh, xps, pe_shift=False))
                    else:
                        b_tail(j, hh, xps, pe_shift=False)

            for j in range(4):
                b_pair_full(j, with_v=(j == 0), last=(j == 3))
            while pending_tail:
                pending_tail.pop(0)()

            # ---- Phase C: output projection ----
            # First groups open with only jj=0..2 accumulated (their x_all
            # slices are long ready), so the PE can run them during the last
            # head's denominator tail; jj=3 closes the group afterwards.
            def c_group_open(t, n):
                ps = ps1024.tile([P, T], f32, name="sc_c", tag="sc")[:, 0:F]
                for jj in range(3):
                    nc.tensor.matmul(
                        ps, lhsT=x_all[:, jj, t * P:(t + 1) * P],
                        rhs=wo_sb[:, jj, n * F:(n + 1) * F],
                        start=(jj == 0), stop=False,
                    )
                return ps

            def c_group_close(t, n, ps):
                nc.tensor.matmul(
                    ps, lhsT=x_all[:, 3, t * P:(t + 1) * P],
                    rhs=wo_sb[:, 3, n * F:(n + 1) * F],
                    start=False, stop=True,
                )
                ob = outp.tile([P, F], out_dt, name="ob", tag="ob")
                nc.scalar.activation(
                    out=ob, in_=ps, func=mybir.ActivationFunctionType.Copy)
                nc.sync.dma_start(out=out_view[:, t, n * F:(n + 1) * F], in_=ob)

            def c_group(t, nsplit=1):
                for n in range(2):
                    ps = ps1024.tile([P, T], f32, name="sc_c", tag="sc")[:, 0:F]
                    for jj in range(4):
                        nc.tensor.matmul(
                            ps, lhsT=x_all[:, jj, t * P:(t + 1) * P],
                            rhs=wo_sb[:, jj, n * F:(n + 1) * F],
                            start=(jj == 0), stop=(jj == 3),
                        )
                    ob = outp.tile([P, F], out_dt, name="ob", tag="ob")
                    if t == 7 and n == 1:
                        # drain the final group on DVE so it runs concurrently
                        # with Act's previous drain, shortening the exposed tail
                        nc.vector.tensor_copy(out=ob, in_=ps)
                    else:
                        nc.scalar.activation(
                            out=ob, in_=ps, func=mybir.ActivationFunctionType.Copy)
                    nc.sync.dma_start(out=out_view[:, t, n * F:(n + 1) * F], in_=ob)

            ps00 = c_group_open(0, 0)
            ps01 = c_group_open(0, 1)
            c_group_close(0, 0, ps00)
            c_group_close(0, 1, ps01)
            for t in range(1, 8):
                c_group(t)

    return nc


def _get_nc(flags):
    if len(flags) == 3:
        flags = (*flags, True)
    key = (flags, NORM_ENGINE, GDRAIN, SC_BUFS, PS512_BUFS, ATTN_BUFS,
           DRAIN_BUFS, STAT_BUFS, QHAT_BUFS, MASK_FULL_T, OUT_DTYPE,
           V_ILV, TAIL_DEFER, PE_SHIFT_LAST, WARMUP_MM)
    if key not in _CACHE:
        nc = _build(*flags)
        patched, _n = _split_excess_waits(nc.to_json_bytes())
        nc.to_json_bytes = lambda: patched
        _CACHE[key] = nc
    return _CACHE[key]


def kernel(query, key, value, mask, Wq, bq, Wk, bk, Wv, bv, Wo, bo,
           q_gamma, q_beta, k_gamma, k_beta, _trace=False):
    from concourse.bass_utils import run_bass_kernel_spmd

    query = np.asarray(query, np.float32)
    key = np.asarray(key, np.float32)
    value = np.asarray(value, np.float32)
    mask = np.asarray(mask)
    Wq, Wk, Wv, Wo = (np.asarray(w, np.float32) for w in (Wq, Wk, Wv, Wo))
    bq, bk, bv, bo = (np.asarray(b, np.float32) for b in (bq, bk, bv, bo))
    q_gamma, q_beta, k_gamma, k_beta = (
        np.asarray(g, np.float32) for g in (q_gamma, q_beta, k_gamma, k_beta))

    B = query.shape[0]
    use_bq, use_bk, use_bv = (bool(np.any(b)) for b in (bq, bk, bv))
    ln_beta_zero = not (np.any(q_beta) or np.any(k_beta))
    nc = _get_nc((use_bq, use_bk, use_bv, ln_beta_zero))

    # host-side shard prep (bf16, transposed)
    xqT = [np.ascontiguousarray(query[b].T.astype(BF16)) for b in range(B)]
    xkT = [np.ascontiguousarray(key[b].T.astype(BF16)) for b in range(B)]
    xvT = [np.ascontiguousarray(value[b].T.astype(BF16)) for b in range(B)]
    maskT = [np.ascontiguousarray((~mask[b]).T.astype(BF16)) for b in range(B)]
    roll64 = np.ascontiguousarray(np.roll(np.eye(P, dtype=np.float32), 64, axis=1))
    gq8 = np.ascontiguousarray((np.tile(q_gamma, 2) / 8.0).reshape(P, 1))
    bq8 = np.ascontiguousarray((np.tile(q_beta, 2) / 8.0).reshape(P, 1))
    gk2 = np.ascontiguousarray(np.tile(k_gamma, 2).reshape(P, 1))
    bk2 = np.ascontiguousarray(np.tile(k_beta, 2).reshape(P, 1))

    in_maps = []
    for core in range(8):
        b, g = core // 2, core % 2
        sl = slice(g * F, (g + 1) * F)
        im = {
            "xq_t": xqT[b], "xk_t": xkT[b], "xv_t": xvT[b],
            "roll_d": roll64,
            "wq_t": np.ascontiguousarray(Wq[sl].T.astype(BF16)),
            "wk_t": np.ascontiguousarray(Wk[sl].T.astype(BF16)),
            "wv_t": np.ascontiguousarray(Wv[sl].T.astype(BF16)),
            "wo_t": np.ascontiguousarray(Wo[:, sl].T.astype(BF16)),
            "mask_t": maskT[b],
            "gq": gq8, "bq_ln": bq8, "gk": gk2, "bk_ln": bk2,
        }
        if use_bq:
            im["bq"] = np.ascontiguousarray(bq[sl])
        if use_bk:
            im["bk"] = np.ascontiguousarray(bk[sl])
        if use_bv:
            im["bv"] = np.ascontiguousarray(bv[sl])
        in_maps.append(im)

    res = run_bass_kernel_spmd(nc, in_maps, core_ids=list(range(8)), trace=_trace)
    out = np.zeros((B, T, D), np.float32)
    for b in range(B):
        out[b] = (np.asarray(res.results[2 * b]["out_p"], np.float32)
                  + np.asarray(res.results[2 * b + 1]["out_p"], np.float32) + bo)
    if _trace:
        kernel._last_results = res
    return out
